# revision 8
# baseline (speedup 1.0000x reference)
"""DBOT Sinkhorn loss kernel for 8 Trainium2 NeuronCores.

Strategy (v2 — all-PE dual-slab)
--------------------------------
S = img @ text.T serves both cross-entropy terms (logits_per_text = S.T).
The Sinkhorn scalings factor as P = diag(u) P0 diag(v), P0 = exp(S-1), so
each iteration only needs matrix-vector products with P0 / P0^T.

Each core k holds TWO fp8 slabs in SBUF (64 KB/partition each):
  slabA = P0[rows Ik, :]      (row block of P0)
  slabB = P0^T[rows Ik, :]    (= column block of P0, transposed)
built by two fp8-DoubleRow gram matmuls (textT / imgT as moving operands).
With both slabs, every Sinkhorn product becomes a PE column-sum mat-vec
(contract over partitions) in fp8 DoubleRow — ~2x the bf16 rate — and the
tensor engine never idles long enough for the HAM clock-gate to rethrottle.

Each product yields an [N] partial; a ReduceScatter hands core k exactly
its local slice, which is also the slice needed for the next stationary
vector.  Stationary vectors are rescaled by their first element and cast
to fp8 (the Sinkhorn recurrence is self-correcting; validated to ~5e-6
final relative error in simulation).

The cross entropy uses that final-plan entries are small (~0.1):
  lse_i = log(N + r_i + r_i^2/2N + r_i^3/6N^2 + (e^d - 1 - d - d^2/2 - d^3/6))
with r_i = u_i (P0 v)_i (one more product) and d = u_i P0_ii v_i (diag from
features).  Host combines per-core [R] outputs in float64.
"""

import sys

sys.path.insert(0, "/opt/trn_rl_repo")

import numpy as np

N = 8192
D = 1024
NC = 8
R = N // NC          # rows per core
P = 128              # SBUF partitions
IB = R // P          # 8 row blocks per core
JT = N // 512        # 16 column tiles of 512
ITERS = 5
BD = 0.1 * N
BU = 0.9 * N
NPROD = 2 + 2 + 4 * (ITERS - 1) + 2   # products: it0 (y,w per chain) + 4/iter + final

_BUILD_CACHE = {}


def _round_fp8(x):
    from concourse import mybir

    np_f8 = mybir.dt.np(mybir.dt.float8e4)
    return np.ascontiguousarray(x, np.float32).astype(np_f8)


def _split_excess_waits(nc, max_waits=1):
    """Walrus CTRL lowering rejects instructions carrying several sem waits.
    Hoist all but the last wait of any multi-wait instruction into dedicated
    NoOps placed immediately before it on the same engine."""
    from concourse import mybir

    for f in nc.m.functions:
        for bb in f.blocks:
            insts = bb.instructions
            new_insts = []
            for inst in insts:
                si = inst.sync_info
                if si and si.on_wait and len(si.on_wait) > max_waits:
                    waits = list(si.on_wait)
                    head, tail = waits[:-max_waits], waits[-max_waits:]
                    for k, w in enumerate(head):
                        nop = mybir.InstNoOp(
                            name=f"{inst.name}-waitsplit-{k}",
                            engine=inst.engine,
                            ins=[],
                            outs=[],
                            sync_info=type(si)(on_wait=[w], on_update=[]),
                        )
                        new_insts.append(nop)
                    inst.sync_info = type(si)(
                        on_wait=tail, on_update=list(si.on_update or [])
                    )
                new_insts.append(inst)
            bb.instructions = new_insts


def _build():
    """Build the Bass module (same SPMD program for all 8 cores)."""
    from contextlib import ExitStack

    import concourse.bass as bass
    import concourse.tile as tile
    from concourse import mybir

    f32 = mybir.dt.float32
    bf16 = mybir.dt.bfloat16
    f8 = mybir.dt.float8e4
    ALU = mybir.AluOpType
    ACTF = mybir.ActivationFunctionType
    DR = mybir.MatmulPerfMode.DoubleRow
    RG = [list(range(NC))]

    nc = bass.Bass("TRN2", target_bir_lowering=False, debug=False, num_devices=NC)

    # ---- external I/O ----
    imgT_l = nc.dram_tensor("imgT_l", [P, 8, R], f8, kind="ExternalInput")
    textT_l = nc.dram_tensor("textT_l", [P, 8, R], f8, kind="ExternalInput")
    textT_g = nc.dram_tensor("textT_g", [P, JT, 8, 512], f8, kind="ExternalInput")
    imgT_g = nc.dram_tensor("imgT_g", [P, JT, 8, 512], f8, kind="ExternalInput")

    out_rA = nc.dram_tensor("out_rA", [P, IB], f32, kind="ExternalOutput")
    out_rB = nc.dram_tensor("out_rB", [P, IB], f32, kind="ExternalOutput")
    out_dA = nc.dram_tensor("out_dA", [P, IB], f32, kind="ExternalOutput")
    out_dB = nc.dram_tensor("out_dB", [P, IB], f32, kind="ExternalOutput")

    # ---- internal DRAM ----
    d0_dram = nc.dram_tensor("d0_dram", [R], f32)
    ps_in = [nc.dram_tensor(f"ps_in{t}", [N], f32) for t in range(NPROD)]
    ps_out = [nc.dram_tensor(f"ps_out{t}", [R], f32) for t in range(NPROD)]

    UMEAN = 3000.0  # ~N * mean(exp(S-1)); fp8 scales only need ~100x accuracy

    with tile.TileContext(nc) as tc, ExitStack() as ctx:
        state = ctx.enter_context(tc.tile_pool(name="state", bufs=1))
        slabA = state.tile([P, IB, JT, 512], f8)
        slabB = state.tile([P, IB, JT, 512], f8)
        statA = state.tile([P, IB, 16], f8)
        statB = state.tile([P, IB, 16], f8)
        ones16 = state.tile([P, 1], bf16)
        negone = state.tile([P, 1], f32)
        d0 = state.tile([P, IB], f32)
        uA = state.tile([P, IB], f32)
        uB = state.tile([P, IB], f32)
        vA = state.tile([P, IB], f32)
        vB = state.tile([P, IB], f32)
        yA = state.tile([P, IB], f32)
        yB = state.tile([P, IB], f32)
        wA = state.tile([P, IB], f32)
        wB = state.tile([P, IB], f32)
        c1 = state.tile([P, IB], f32)
        c2 = state.tile([P, IB], f32)
        c3 = state.tile([P, IB], f32)

        nc.vector.memset(ones16, 1.0)
        nc.vector.memset(negone, -1.0)
        nc.vector.memset(statA, 1.0)
        nc.vector.memset(statB, 1.0)
        nc.vector.memset(vA, 1.0)
        nc.vector.memset(vB, 1.0)

        # ============ feature load + diag pre-phase ============
        featp = ctx.enter_context(tc.tile_pool(name="featp", bufs=1))
        imgT_sb = featp.tile([P, 8, R], f8)
        textTl_sb = featp.tile([P, 8, R], f8)
        nc.sync.dma_start(out=imgT_sb[:], in_=imgT_l.ap())
        nc.sync.dma_start(out=textTl_sb[:], in_=textT_l.ap())

        with (
            tc.tile_pool(name="prep", bufs=1) as prep,
            tc.tile_pool(name="preps", bufs=1, space="PSUM") as preps,
        ):
            prodD = prep.tile([P, 8, R], bf16)
            nc.vector.tensor_mul(prodD[:], imgT_sb[:], textTl_sb[:])
            ps_d = preps.tile([1, 2, 512], f32)
            for h in range(2):
                for db in range(8):
                    nc.tensor.matmul(
                        ps_d[0:1, h, :],
                        ones16[:],
                        prodD[:, db, h * 512 : (h + 1) * 512],
                        start=(db == 0),
                        stop=(db == 7),
                    )
            sd = prep.tile([1, R], f32)
            nc.scalar.activation(
                sd[0:1, :], ps_d[0:1, :, :], ACTF.Exp, bias=negone[0:1, :]
            )
            nc.sync.dma_start(out=d0_dram.ap(), in_=sd[0:1, :])
        nc.gpsimd.dma_start(
            out=d0[:], in_=d0_dram.ap().rearrange("(ib p) -> p ib", p=P)
        )

        # ============ product pools (before gram pools: LIFO close order) ============
        pps = ctx.enter_context(tc.tile_pool(name="pps", bufs=2, space="PSUM"))
        stgp = ctx.enter_context(tc.tile_pool(name="stgp", bufs=2))

        # ============ gram phase: slabA = exp(S-1), slabB = exp(S.T-1) ============
        gram_ctx = ExitStack()
        mvp = gram_ctx.enter_context(tc.tile_pool(name="mvp", bufs=2))
        gps = gram_ctx.enter_context(tc.tile_pool(name="gps", bufs=3, space="PSUM"))

        def gram(stat_sb, mv_d, slab):
            for jc in range(4):
                mv = mvp.tile([P, 4, 8, 512], f8, tag="mv")
                nc.sync.dma_start(out=mv[:], in_=mv_d.ap()[:, jc * 4 : (jc + 1) * 4, :, :])
                for ib in range(8):
                    for half in range(2):
                        ps = gps.tile([P, 2, 512], f32, tag="gps")
                        for jl in range(2):
                            for db in range(4):
                                nc.tensor.matmul(
                                    ps[:, jl, :],
                                    stat_sb[:, db * 2 : db * 2 + 2, ib * P : (ib + 1) * P],
                                    mv[:, half * 2 + jl, db * 2 : db * 2 + 2, :],
                                    start=(db == 0),
                                    stop=(db == 3),
                                    perf_mode=DR,
                                )
                        nc.scalar.activation(
                            slab[:, ib, jc * 4 + half * 2 : jc * 4 + half * 2 + 2, :],
                            ps[:],
                            ACTF.Exp,
                            bias=negone[:],
                        )

        # ============ product machinery ============
        def product(slab, stat, rescale, t):
            """ps_out[t] <- RS over cores of rescale * [N]-partial of (stat . slab)."""
            for jh in range(4):
                stg = stgp.tile([1, 4, 512], f32, tag="stg")
                for jl in range(4):
                    jt = jh * 4 + jl
                    ps = pps.tile([1, 512], f32, tag="pps")
                    for q in range(4):
                        nc.tensor.matmul(
                            ps[0:1, :],
                            stat[:, 2 * q : 2 * q + 2, 0:1],
                            slab[:, 2 * q : 2 * q + 2, jt, :],
                            start=(q == 0),
                            stop=(q == 3),
                            perf_mode=DR,
                        )
                    nc.scalar.activation(
                        stg[0:1, jl, :], ps[0:1, :], ACTF.Copy, scale=float(rescale)
                    )
                nc.sync.dma_start(
                    out=ps_in[t].ap()[jh * 2048 : (jh + 1) * 2048], in_=stg[0:1, :, :]
                )
            nc.gpsimd.collective_compute(
                "ReduceScatter", ALU.add, replica_groups=RG,
                ins=[ps_in[t].ap()], outs=[ps_out[t].ap()],
            )

        def recv_y(t, y, u, stat, qscale):
            """u = 1/y; stat = f8(u * qscale)."""
            nc.gpsimd.dma_start(out=y[:], in_=ps_out[t].ap().rearrange("(ib p) -> p ib", p=P))
            nc.vector.reciprocal(u[:], y[:])
            nc.vector.tensor_scalar_mul(stat[:, :, 0:1], u[:], float(qscale))

        def recv_w(t, w, v, stat, qscale):
            """colstep: v *= max(BD/c,1)*min(BU/(c*f1),1), c = v.w; stat = f8(v*qscale)."""
            nc.gpsimd.dma_start(out=w[:], in_=ps_out[t].ap().rearrange("(ib p) -> p ib", p=P))
            nc.vector.tensor_mul(c1[:], v[:], w[:])
            nc.vector.reciprocal(c2[:], c1[:])
            nc.vector.tensor_scalar(c2[:], c2[:], BD, 1.0, op0=ALU.mult, op1=ALU.max)
            nc.vector.tensor_mul(c3[:], c1[:], c2[:])
            nc.vector.tensor_mul(v[:], v[:], c2[:])
            nc.vector.reciprocal(c1[:], c3[:])
            nc.vector.tensor_scalar(c1[:], c1[:], BU, 1.0, op0=ALU.mult, op1=ALU.min)
            nc.vector.tensor_mul(v[:], v[:], c1[:])
            nc.vector.tensor_scalar_mul(stat[:, :, 0:1], v[:], float(qscale))

        # ============ gram + Sinkhorn, pipelined ============
        # Emission rule: each product's recv follows it IMMEDIATELY so its
        # gpsimd recv-DMA precedes later RS triggers in the queue, and every
        # stationary dependency has one full product (~16us) of PE cover.
        # it0 slots: t0 = y_B (over slabA), t1 = y_A, t2 = w_B, t3 = w_A
        gram(imgT_sb, textT_g, slabA)
        product(slabA, statB, 1.0, 0)              # y_B partial; RS under gramB
        gram(textTl_sb, imgT_g, slabB)
        recv_y(0, yB, uB, statB, UMEAN)
        product(slabB, statA, 1.0, 1)              # y_A
        recv_y(1, yA, uA, statA, UMEAN)
        product(slabB, statB, 1.0 / UMEAN, 2)      # w_B = P0 u_B
        recv_w(2, wB, vB, statB, 1.0 / BD)
        product(slabA, statA, 1.0 / UMEAN, 3)      # w_A = P0^T u_A
        recv_w(3, wA, vA, statA, 1.0 / BD)
        gram_ctx.close()

        t = 4
        for it in range(1, ITERS):
            sv = BD ** it          # v magnitude entering this iteration
            su = UMEAN * BD ** it  # 1/u magnitude this iteration
            product(slabA, statB, sv, t)           # y_B = P0^T v_B
            recv_y(t, yB, uB, statB, su)
            product(slabB, statA, sv, t + 1)       # y_A = P0 v_A
            recv_y(t + 1, yA, uA, statA, su)
            product(slabB, statB, 1.0 / su, t + 2)  # w_B = P0 u_B
            recv_w(t + 2, wB, vB, statB, 1.0 / BD ** (it + 1))
            product(slabA, statA, 1.0 / su, t + 3)  # w_A = P0^T u_A
            recv_w(t + 3, wA, vA, statA, 1.0 / BD ** (it + 1))
            t += 4

        # d outputs don't depend on the final products: emit early
        nc.vector.tensor_mul(c3[:], uA[:], d0[:])
        nc.vector.tensor_mul(c3[:], c3[:], vA[:])
        nc.sync.dma_start(out=out_dA.ap(), in_=c3[:])
        nc.vector.tensor_mul(c2[:], uB[:], d0[:])
        nc.vector.tensor_mul(c2[:], c2[:], vB[:])
        nc.sync.dma_start(out=out_dB.ap(), in_=c2[:])

        # final row-sum products for the cross entropy
        sv = BD ** ITERS
        tA, tB = t + 1, t
        product(slabA, statB, sv, tB)              # y6_B = P0^T v_B5
        product(slabB, statA, sv, tA)              # y6_A = P0 v_A5
        nc.gpsimd.dma_start(out=yB[:], in_=ps_out[tB].ap().rearrange("(ib p) -> p ib", p=P))
        nc.gpsimd.dma_start(out=yA[:], in_=ps_out[tA].ap().rearrange("(ib p) -> p ib", p=P))

        # outputs: r = u.y6
        nc.vector.tensor_mul(c1[:], uA[:], yA[:])
        nc.sync.dma_start(out=out_rA.ap(), in_=c1[:])
        nc.vector.tensor_mul(c2[:], uB[:], yB[:])
        nc.sync.dma_start(out=out_rB.ap(), in_=c2[:])

    _split_excess_waits(nc)
    return nc


def _get_nc():
    if "nc" not in _BUILD_CACHE:
        _BUILD_CACHE["nc"] = _build()
    return _BUILD_CACHE["nc"]


def _fallback(img, txt, labels):
    """Reference math on host (only for unexpected label patterns)."""
    S = img.astype(np.float64) @ txt.astype(np.float64).T

    def sink(Pin):
        n = Pin.shape[0]
        Pm = np.exp(-Pin)
        for _ in range(ITERS):
            Pm = (1.0 / Pm.sum(1))[:, None] * Pm
            Pm = Pm * np.maximum(BD / Pm.sum(0), 1.0)[None, :]
            Pm = Pm * np.minimum(BU / Pm.sum(0), 1.0)[None, :]
        return Pm

    def ce(logits, lab):
        m = logits.max(1, keepdims=True)
        lse = np.log(np.exp(logits - m).sum(1)) + m[:, 0]
        picked = logits[np.arange(logits.shape[0]), lab]
        return np.mean(lse - picked)

    lab = np.asarray(labels, np.int64)
    loss = 0.5 * (ce(sink(1.0 - S), lab) + ce(sink(1.0 - S.T), lab))
    return np.float32(loss)


def kernel(all_image_features, all_text_features, logit_scale, labels):
    from concourse.bass_utils import run_bass_kernel_spmd

    img = np.ascontiguousarray(np.asarray(all_image_features), np.float32)
    txt = np.ascontiguousarray(np.asarray(all_text_features), np.float32)
    lab = np.asarray(labels)
    assert img.shape == (N, D) and txt.shape == (N, D)
    if not np.array_equal(lab.astype(np.int64), np.arange(N, dtype=np.int64)):
        return _fallback(img, txt, lab)

    img8 = _round_fp8(img)
    txt8 = _round_fp8(txt)

    # DoubleRow layout: dim g = db*2 + c maps to d = db*256 + c*128 + p.
    # moving:    X_g[p, jt, g, j] = x[jt*512 + j, d(g, p)]
    # stationary X_l[p, g, i]    = x[block_k][i, d(g, p)]
    def moving(x8):
        return np.ascontiguousarray(
            x8.reshape(JT, 512, 4, 2, P).transpose(4, 0, 2, 3, 1).reshape(P, JT, 8, 512)
        )

    def stationary(x8):
        return np.ascontiguousarray(
            x8.reshape(R, 4, 2, P).transpose(3, 1, 2, 0).reshape(P, 8, R)
        )

    textT_g = moving(txt8)
    imgT_g = moving(img8)
    in_maps = []
    for k in range(NC):
        sl = slice(k * R, (k + 1) * R)
        in_maps.append({
            "imgT_l": stationary(img8[sl]),
            "textT_l": stationary(txt8[sl]),
            "textT_g": textT_g,
            "imgT_g": imgT_g,
        })

    nc = _get_nc()
    _BUILD_CACHE["in_maps"] = in_maps
    res = run_bass_kernel_spmd(nc, in_maps, list(range(NC)))

    # ---- host-side combine (O(N) work, float64) ----
    def gather(name):
        return np.concatenate(
            [res.results[k][name].astype(np.float64).T.reshape(R) for k in range(NC)]
        )

    rA, rB = gather("out_rA"), gather("out_rB")
    dA, dB = gather("out_dA"), gather("out_dB")

    def ce_loss(r, d):
        gd = np.exp(d) - 1.0 - d - d * d / 2.0 - d ** 3 / 6.0
        lse = np.log(N + r + r * r / (2.0 * N) + r ** 3 / (6.0 * N * N) + gd)
        return np.mean(lse - d)

    return np.float32(0.5 * (ce_loss(rA, dA) + ce_loss(rB, dB)))


# revision 9
# speedup vs baseline: 1.2313x; 1.2313x over previous
"""DBOT Sinkhorn loss kernel for 8 Trainium2 NeuronCores.

Strategy (v2 — all-PE dual-slab)
--------------------------------
S = img @ text.T serves both cross-entropy terms (logits_per_text = S.T).
The Sinkhorn scalings factor as P = diag(u) P0 diag(v), P0 = exp(S-1), so
each iteration only needs matrix-vector products with P0 / P0^T.

Each core k holds TWO fp8 slabs in SBUF (64 KB/partition each):
  slabA = P0[rows Ik, :]      (row block of P0)
  slabB = P0^T[rows Ik, :]    (= column block of P0, transposed)
built by two fp8-DoubleRow gram matmuls (textT / imgT as moving operands).
With both slabs, every Sinkhorn product becomes a PE column-sum mat-vec
(contract over partitions) in fp8 DoubleRow — ~2x the bf16 rate — and the
tensor engine never idles long enough for the HAM clock-gate to rethrottle.

Each product yields an [N] partial; a ReduceScatter hands core k exactly
its local slice, which is also the slice needed for the next stationary
vector.  Stationary vectors are rescaled by their first element and cast
to fp8 (the Sinkhorn recurrence is self-correcting; validated to ~5e-6
final relative error in simulation).

The cross entropy uses that final-plan entries are small (~0.1):
  lse_i = log(N + r_i + r_i^2/2N + r_i^3/6N^2 + (e^d - 1 - d - d^2/2 - d^3/6))
with r_i = u_i (P0 v)_i (one more product) and d = u_i P0_ii v_i (diag from
features).  Host combines per-core [R] outputs in float64.
"""

import sys

sys.path.insert(0, "/opt/trn_rl_repo")

import numpy as np

N = 8192
D = 1024
NC = 8
R = N // NC          # rows per core
P = 128              # SBUF partitions
IB = R // P          # 8 row blocks per core
JT = N // 512        # 16 column tiles of 512
ITERS = 5
BD = 0.1 * N
BU = 0.9 * N
NPROD = 2 + 2 + 4 * (ITERS - 1) + 2   # products: it0 (y,w per chain) + 4/iter + final

_BUILD_CACHE = {}


def _round_fp8(x):
    from concourse import mybir

    np_f8 = mybir.dt.np(mybir.dt.float8e4)
    return np.ascontiguousarray(x, np.float32).astype(np_f8)


def _split_excess_waits(nc, max_waits=1):
    """Walrus CTRL lowering rejects instructions carrying several sem waits.
    Hoist all but the last wait of any multi-wait instruction into dedicated
    NoOps placed immediately before it on the same engine."""
    from concourse import mybir

    for f in nc.m.functions:
        for bb in f.blocks:
            insts = bb.instructions
            new_insts = []
            for inst in insts:
                si = inst.sync_info
                if si and si.on_wait and len(si.on_wait) > max_waits:
                    waits = list(si.on_wait)
                    head, tail = waits[:-max_waits], waits[-max_waits:]
                    for k, w in enumerate(head):
                        nop = mybir.InstNoOp(
                            name=f"{inst.name}-waitsplit-{k}",
                            engine=inst.engine,
                            ins=[],
                            outs=[],
                            sync_info=type(si)(on_wait=[w], on_update=[]),
                        )
                        new_insts.append(nop)
                    inst.sync_info = type(si)(
                        on_wait=tail, on_update=list(si.on_update or [])
                    )
                new_insts.append(inst)
            bb.instructions = new_insts


def _build():
    """Build the Bass module (same SPMD program for all 8 cores)."""
    from contextlib import ExitStack

    import concourse.bass as bass
    import concourse.tile as tile
    from concourse import mybir

    f32 = mybir.dt.float32
    bf16 = mybir.dt.bfloat16
    f8 = mybir.dt.float8e4
    ALU = mybir.AluOpType
    ACTF = mybir.ActivationFunctionType
    DR = mybir.MatmulPerfMode.DoubleRow
    RG = [list(range(NC))]

    nc = bass.Bass("TRN2", target_bir_lowering=False, debug=False, num_devices=NC)

    # ---- external I/O ----
    imgT_l = nc.dram_tensor("imgT_l", [P, 8, R], f8, kind="ExternalInput")
    textT_l = nc.dram_tensor("textT_l", [P, 8, R], f8, kind="ExternalInput")
    textT_g = nc.dram_tensor("textT_g", [P, JT, 8, 512], f8, kind="ExternalInput")
    imgT_g = nc.dram_tensor("imgT_g", [P, JT, 8, 512], f8, kind="ExternalInput")

    out_rA = nc.dram_tensor("out_rA", [P, IB], f32, kind="ExternalOutput")
    out_rB = nc.dram_tensor("out_rB", [P, IB], f32, kind="ExternalOutput")
    out_dA = nc.dram_tensor("out_dA", [P, IB], f32, kind="ExternalOutput")
    out_dB = nc.dram_tensor("out_dB", [P, IB], f32, kind="ExternalOutput")

    # ---- internal DRAM ----
    d0_dram = nc.dram_tensor("d0_dram", [R], f32)
    ps_in = [nc.dram_tensor(f"ps_in{t}", [N], f32) for t in range(NPROD)]
    ps_out = [nc.dram_tensor(f"ps_out{t}", [R], f32) for t in range(NPROD)]

    UMEAN = 3000.0  # ~N * mean(exp(S-1)); fp8 scales only need ~100x accuracy

    with tile.TileContext(nc) as tc, ExitStack() as ctx:
        state = ctx.enter_context(tc.tile_pool(name="state", bufs=1))
        slabA = state.tile([P, IB, JT, 512], f8)
        slabB = state.tile([P, IB, JT, 512], f8)
        statA = state.tile([P, IB, 16], f8)
        statB = state.tile([P, IB, 16], f8)
        ones16 = state.tile([P, 1], bf16)
        negone = state.tile([P, 1], f32)
        d0 = state.tile([P, IB], f32)
        uA = state.tile([P, IB], f32)
        uB = state.tile([P, IB], f32)
        vA = state.tile([P, IB], f32)
        vB = state.tile([P, IB], f32)
        yA = state.tile([P, IB], f32)
        yB = state.tile([P, IB], f32)
        wA = state.tile([P, IB], f32)
        wB = state.tile([P, IB], f32)
        c1 = state.tile([P, IB], f32)
        c2 = state.tile([P, IB], f32)
        c3 = state.tile([P, IB], f32)

        nc.vector.memset(ones16, 1.0)
        nc.vector.memset(negone, -1.0)
        nc.vector.memset(statA, 1.0)
        nc.vector.memset(statB, 1.0)
        nc.vector.memset(vA, 1.0)
        nc.vector.memset(vB, 1.0)

        # ============ feature load + diag pre-phase ============
        featp = ctx.enter_context(tc.tile_pool(name="featp", bufs=1))
        imgT_sb = featp.tile([P, 8, R], f8)
        textTl_sb = featp.tile([P, 8, R], f8)
        nc.sync.dma_start(out=imgT_sb[:], in_=imgT_l.ap())
        nc.sync.dma_start(out=textTl_sb[:], in_=textT_l.ap())

        with (
            tc.tile_pool(name="prep", bufs=1) as prep,
            tc.tile_pool(name="preps", bufs=1, space="PSUM") as preps,
        ):
            prodD = prep.tile([P, 8, R], bf16)
            nc.vector.tensor_mul(prodD[:], imgT_sb[:], textTl_sb[:])
            ps_d = preps.tile([1, 2, 512], f32)
            for h in range(2):
                for db in range(8):
                    nc.tensor.matmul(
                        ps_d[0:1, h, :],
                        ones16[:],
                        prodD[:, db, h * 512 : (h + 1) * 512],
                        start=(db == 0),
                        stop=(db == 7),
                    )
            sd = prep.tile([1, R], f32)
            nc.scalar.activation(
                sd[0:1, :], ps_d[0:1, :, :], ACTF.Exp, bias=negone[0:1, :]
            )
            nc.sync.dma_start(out=d0_dram.ap(), in_=sd[0:1, :])
        nc.gpsimd.dma_start(
            out=d0[:], in_=d0_dram.ap().rearrange("(ib p) -> p ib", p=P)
        )

        # ============ product pools (before gram pools: LIFO close order) ============
        pps = ctx.enter_context(tc.tile_pool(name="pps", bufs=2, space="PSUM"))
        stgp = ctx.enter_context(tc.tile_pool(name="stgp", bufs=2))

        # ============ gram phase: slabA = exp(S-1), slabB = exp(S.T-1) ============
        gram_ctx = ExitStack()
        mvp = gram_ctx.enter_context(tc.tile_pool(name="mvp", bufs=2))
        gps = gram_ctx.enter_context(tc.tile_pool(name="gps", bufs=3, space="PSUM"))

        def gram(stat_sb, mv_d, slab):
            for jc in range(4):
                mv = mvp.tile([P, 4, 8, 512], f8, tag="mv")
                nc.sync.dma_start(out=mv[:], in_=mv_d.ap()[:, jc * 4 : (jc + 1) * 4, :, :])
                for ib in range(8):
                    for half in range(2):
                        ps = gps.tile([P, 2, 512], f32, tag="gps")
                        for jl in range(2):
                            for db in range(4):
                                nc.tensor.matmul(
                                    ps[:, jl, :],
                                    stat_sb[:, db * 2 : db * 2 + 2, ib * P : (ib + 1) * P],
                                    mv[:, half * 2 + jl, db * 2 : db * 2 + 2, :],
                                    start=(db == 0),
                                    stop=(db == 3),
                                    perf_mode=DR,
                                )
                        nc.scalar.activation(
                            slab[:, ib, jc * 4 + half * 2 : jc * 4 + half * 2 + 2, :],
                            ps[:],
                            ACTF.Exp,
                            bias=negone[:],
                        )

        # ============ product machinery ============
        def product(slab, stat, rescale, t):
            """ps_out[t] <- RS over cores of rescale * [N]-partial of (stat . slab)."""
            for jh in range(4):
                stg = stgp.tile([1, 4, 512], f32, tag="stg")
                for jl in range(4):
                    jt = jh * 4 + jl
                    ps = pps.tile([1, 512], f32, tag="pps")
                    for q in range(4):
                        nc.tensor.matmul(
                            ps[0:1, :],
                            stat[:, 2 * q : 2 * q + 2, 0:1],
                            slab[:, 2 * q : 2 * q + 2, jt, :],
                            start=(q == 0),
                            stop=(q == 3),
                            perf_mode=DR,
                        )
                    nc.scalar.activation(
                        stg[0:1, jl, :], ps[0:1, :], ACTF.Copy, scale=float(rescale)
                    )
                nc.scalar.dma_start(
                    out=ps_in[t].ap()[jh * 2048 : (jh + 1) * 2048], in_=stg[0:1, :, :]
                )
            nc.gpsimd.collective_compute(
                "ReduceScatter", ALU.add, replica_groups=RG,
                ins=[ps_in[t].ap()], outs=[ps_out[t].ap()],
            )

        def recv_y(t, y, u, stat, qscale):
            """u = 1/y; stat = f8(u * qscale)."""
            nc.sync.dma_start(out=y[:], in_=ps_out[t].ap().rearrange("(ib p) -> p ib", p=P))
            nc.vector.reciprocal(u[:], y[:])
            nc.vector.tensor_scalar_mul(stat[:, :, 0:1], u[:], float(qscale))

        def recv_w(t, w, v, stat, qscale):
            """colstep: v *= max(BD/c,1)*min(BU/(c*f1),1), c = v.w; stat = f8(v*qscale)."""
            nc.sync.dma_start(out=w[:], in_=ps_out[t].ap().rearrange("(ib p) -> p ib", p=P))
            nc.vector.tensor_mul(c1[:], v[:], w[:])
            nc.vector.reciprocal(c2[:], c1[:])
            nc.vector.tensor_scalar(c2[:], c2[:], BD, 1.0, op0=ALU.mult, op1=ALU.max)
            nc.vector.tensor_mul(c3[:], c1[:], c2[:])
            nc.vector.tensor_mul(v[:], v[:], c2[:])
            nc.vector.reciprocal(c1[:], c3[:])
            nc.vector.tensor_scalar(c1[:], c1[:], BU, 1.0, op0=ALU.mult, op1=ALU.min)
            nc.vector.tensor_mul(v[:], v[:], c1[:])
            nc.vector.tensor_scalar_mul(stat[:, :, 0:1], v[:], float(qscale))

        # ============ gram + Sinkhorn, pipelined ============
        # Emission rule: each product's recv follows it IMMEDIATELY so its
        # gpsimd recv-DMA precedes later RS triggers in the queue, and every
        # stationary dependency has one full product (~16us) of PE cover.
        # it0 slots: t0 = y_B (over slabA), t1 = y_A, t2 = w_B, t3 = w_A
        gram(imgT_sb, textT_g, slabA)
        product(slabA, statB, 1.0, 0)              # y_B partial; RS under gramB
        gram(textTl_sb, imgT_g, slabB)
        recv_y(0, yB, uB, statB, UMEAN)
        product(slabB, statA, 1.0, 1)              # y_A
        product(slabB, statB, 1.0 / UMEAN, 2)      # w_B = P0 u_B
        recv_y(1, yA, uA, statA, UMEAN)
        product(slabA, statA, 1.0 / UMEAN, 3)      # w_A = P0^T u_A
        recv_w(2, wB, vB, statB, 1.0 / BD)
        recv_w(3, wA, vA, statA, 1.0 / BD)
        gram_ctx.close()

        t = 4
        for it in range(1, ITERS):
            sv = BD ** it          # v magnitude entering this iteration
            su = UMEAN * BD ** it  # 1/u magnitude this iteration
            product(slabA, statB, sv, t)           # y_B = P0^T v_B
            product(slabB, statA, sv, t + 1)       # y_A = P0 v_A
            recv_y(t, yB, uB, statB, su)
            product(slabB, statB, 1.0 / su, t + 2)  # w_B = P0 u_B
            recv_y(t + 1, yA, uA, statA, su)
            product(slabA, statA, 1.0 / su, t + 3)  # w_A = P0^T u_A
            recv_w(t + 2, wB, vB, statB, 1.0 / BD ** (it + 1))
            recv_w(t + 3, wA, vA, statA, 1.0 / BD ** (it + 1))
            t += 4

        # d outputs don't depend on the final products: emit early
        nc.vector.tensor_mul(c3[:], uA[:], d0[:])
        nc.vector.tensor_mul(c3[:], c3[:], vA[:])
        nc.sync.dma_start(out=out_dA.ap(), in_=c3[:])
        nc.vector.tensor_mul(c2[:], uB[:], d0[:])
        nc.vector.tensor_mul(c2[:], c2[:], vB[:])
        nc.sync.dma_start(out=out_dB.ap(), in_=c2[:])

        # final row-sum products for the cross entropy
        sv = BD ** ITERS
        tA, tB = t + 1, t
        product(slabA, statB, sv, tB)              # y6_B = P0^T v_B5
        product(slabB, statA, sv, tA)              # y6_A = P0 v_A5
        nc.sync.dma_start(out=yB[:], in_=ps_out[tB].ap().rearrange("(ib p) -> p ib", p=P))
        nc.sync.dma_start(out=yA[:], in_=ps_out[tA].ap().rearrange("(ib p) -> p ib", p=P))

        # outputs: r = u.y6
        nc.vector.tensor_mul(c1[:], uA[:], yA[:])
        nc.sync.dma_start(out=out_rA.ap(), in_=c1[:])
        nc.vector.tensor_mul(c2[:], uB[:], yB[:])
        nc.sync.dma_start(out=out_rB.ap(), in_=c2[:])

    _split_excess_waits(nc)
    return nc


def _get_nc():
    if "nc" not in _BUILD_CACHE:
        _BUILD_CACHE["nc"] = _build()
    return _BUILD_CACHE["nc"]


def _fallback(img, txt, labels):
    """Reference math on host (only for unexpected label patterns)."""
    S = img.astype(np.float64) @ txt.astype(np.float64).T

    def sink(Pin):
        n = Pin.shape[0]
        Pm = np.exp(-Pin)
        for _ in range(ITERS):
            Pm = (1.0 / Pm.sum(1))[:, None] * Pm
            Pm = Pm * np.maximum(BD / Pm.sum(0), 1.0)[None, :]
            Pm = Pm * np.minimum(BU / Pm.sum(0), 1.0)[None, :]
        return Pm

    def ce(logits, lab):
        m = logits.max(1, keepdims=True)
        lse = np.log(np.exp(logits - m).sum(1)) + m[:, 0]
        picked = logits[np.arange(logits.shape[0]), lab]
        return np.mean(lse - picked)

    lab = np.asarray(labels, np.int64)
    loss = 0.5 * (ce(sink(1.0 - S), lab) + ce(sink(1.0 - S.T), lab))
    return np.float32(loss)


def kernel(all_image_features, all_text_features, logit_scale, labels):
    from concourse.bass_utils import run_bass_kernel_spmd

    img = np.ascontiguousarray(np.asarray(all_image_features), np.float32)
    txt = np.ascontiguousarray(np.asarray(all_text_features), np.float32)
    lab = np.asarray(labels)
    assert img.shape == (N, D) and txt.shape == (N, D)
    if not np.array_equal(lab.astype(np.int64), np.arange(N, dtype=np.int64)):
        return _fallback(img, txt, lab)

    img8 = _round_fp8(img)
    txt8 = _round_fp8(txt)

    # DoubleRow layout: dim g = db*2 + c maps to d = db*256 + c*128 + p.
    # moving:    X_g[p, jt, g, j] = x[jt*512 + j, d(g, p)]
    # stationary X_l[p, g, i]    = x[block_k][i, d(g, p)]
    def moving(x8):
        return np.ascontiguousarray(
            x8.reshape(JT, 512, 4, 2, P).transpose(4, 0, 2, 3, 1).reshape(P, JT, 8, 512)
        )

    def stationary(x8):
        return np.ascontiguousarray(
            x8.reshape(R, 4, 2, P).transpose(3, 1, 2, 0).reshape(P, 8, R)
        )

    textT_g = moving(txt8)
    imgT_g = moving(img8)
    in_maps = []
    for k in range(NC):
        sl = slice(k * R, (k + 1) * R)
        in_maps.append({
            "imgT_l": stationary(img8[sl]),
            "textT_l": stationary(txt8[sl]),
            "textT_g": textT_g,
            "imgT_g": imgT_g,
        })

    nc = _get_nc()
    _BUILD_CACHE["in_maps"] = in_maps
    res = run_bass_kernel_spmd(nc, in_maps, list(range(NC)))

    # ---- host-side combine (O(N) work, float64) ----
    def gather(name):
        return np.concatenate(
            [res.results[k][name].astype(np.float64).T.reshape(R) for k in range(NC)]
        )

    rA, rB = gather("out_rA"), gather("out_rB")
    dA, dB = gather("out_dA"), gather("out_dB")

    def ce_loss(r, d):
        gd = np.exp(d) - 1.0 - d - d * d / 2.0 - d ** 3 / 6.0
        lse = np.log(N + r + r * r / (2.0 * N) + r ** 3 / (6.0 * N * N) + gd)
        return np.mean(lse - d)

    return np.float32(0.5 * (ce_loss(rA, dA) + ce_loss(rB, dB)))
